# revision 2
# baseline (speedup 1.0000x reference)
"""Trainium2 Bass kernel for nn_AtLocPlusCriterion_VO (optimized).

loss = exp(-srx)*mean|vo_t - tg_t| + srx + exp(-srq)*mean|vo_q - tg_q| + srq
with vo = calc_vo_logq(pred[:-1], pred[1:]).

Changes vs v1 baseline:
- Host folds k = exp(srq-srx) into ALL t-lane planes (pred-t and targ-t).
  vo_t is linear in t, so the t-loss stream is uniformly k-scaled and both
  L1 streams accumulate into ONE accumulator: loss =
  exp(-srq)*(sum|k*dt| + sum|dq|)/(3N) + srx + srq.
- dff subtract and Abs-accumulate merged across t/q streams: one 6C TT +
  one 6C Abs act per tile (adjacent [g2c|x12] vs [g1'|gtq] layouts).
- w1 products merged to one 6C TT; w1/cr combine subs merged to one 6C TT
  via a 12-slab product buffer.
- All tensor work on VectorE (GpSimd tensor ops steal the SBUF port and
  slow VectorE 2-4x -- measured).
- |qV|^2 = 4 - qs2^2 (unit quaternions): kills the 3C Square act and the
  2C add tree; qs2 clamped to +-1.99 via a 4x-mode tensor_scalar.
- Act chain reordered: rowTR(t+1) before pairTR(t) so the next tile's row
  trig does not wait on this tile's arctan chain.
- Row head of tile 0 chunked in halves to cut pipeline-head latency.
- Targ DMA'd into one 9C tile [gtt | g1' | gtq] so g1'/gtq are adjacent.
"""
import os
import numpy as np

N_CORES = 8
T_FULL = 2_000_000
NPAIRS = T_FULL - 1          # 1_999_999
D = 1956                     # pairs per partition per core
C = 978                      # pairs per tile (2 tiles)
NT = 2
R = C + 1                    # rows per tile (halo)
R2 = R + 1                   # padded slab pitch (even)
PPC = 128 * D                # 250_368 pairs per core
PAIRS_PAD = N_CORES * PPC    # 2_002_944
ROWS_PAD = PAIRS_PAD + 1

PL = PPC + 1                 # pred plane length
PT = PPC                     # targ plane length

LN2 = float(np.log(2.0))
LN2SQ2 = float(np.log(2.0 * np.sqrt(2.0)))   # i2n carries 2*sqrt2
PI2 = float(np.pi / 2.0)
SQ2 = float(np.sqrt(2.0))
QCLAMP = 1.984375            # bf16-exact clamp for qs2

_BUILT = {}


def _patch_act_tables():
    import concourse.bacc as bacc_mod
    import concourse.hw_specs as hw

    if getattr(bacc_mod, "_vo_tables_patched", False):
        return
    orig = hw.get_activation_tables

    def steered(arch, _orig=orig):
        from concourse import mybir as _mb
        AF = _mb.ActivationFunctionType
        t = {k: set(v) for k, v in _orig(arch).items()}
        t.get("natural_log", set()).discard(AF.Ln)
        t.get("exp_and_others", set()).discard(AF.Exp)
        t.get("sigmoid_and_others", set()).discard(AF.Arctan)
        return t

    bacc_mod.get_activation_tables = steered
    bacc_mod._vo_tables_patched = True


def _build():
    from concourse import bacc, tile, mybir
    from concourse.ap import AP
    from concourse.bass import _add_dep_helper

    _patch_act_tables()

    f32, bf16 = mybir.dt.float32, mybir.dt.bfloat16
    OP = mybir.AluOpType
    AF = mybir.ActivationFunctionType

    nc = bacc.Bacc("TRN2", target_bir_lowering=False, debug=False,
                   num_devices=N_CORES)
    # pred = [tv(t0)|tv(t1)|tt(t0)|tt(t1)], each tile block 128*(3R) elems,
    # per-partition contiguous 3R-run; targ = [t0|t1] blocks of 128*(6C).
    pred_h = nc.declare_dram_parameter("pred", [NT * 128 * 3 * R * 2], bf16,
                                       isOutput=False)
    targ_h = nc.declare_dram_parameter("targ", [NT * 128 * 6 * C], bf16,
                                       isOutput=False)
    out_h = nc.declare_dram_parameter("out", [128, 4], f32, isOutput=True)

    for v in (PI2, 4.0):
        v = float(v)
        if (f32, v) not in nc.const_aps.aps:
            t = nc.alloc_sbuf_tensor(f"uconst-{v}", [128, 1], f32)
            nc.gpsimd.memset(t.ap(), v)
            nc.const_aps.aps[(f32, v)] = t.ap()
    nc.all_engine_barrier(sem_only=True)

    def sb(tile_, off, dims):
        base = tile_[:, :]
        return AP(base.tensor, base.offset + off,
                  [[base.ap.to_list()[0][0], 128]] + dims)

    accs = []
    groups = {}

    with tile.TileContext(nc) as tc:

        def mkact(tile_i, group, *args, **kw):
            ins = nc.scalar.activation(*args, **kw)
            if group is not None:
                groups.setdefault((tile_i, group), []).append(ins)
            return ins

        with (
            tc.tile_pool(name="inp", bufs=2) as pin,
            tc.tile_pool(name="rowp", bufs=2) as prow,
            tc.tile_pool(name="scr", bufs=1) as pscr,
            tc.tile_pool(name="accp", bufs=4) as pacc,
        ):
            TT = nc.vector.tensor_tensor
            state = {}

            dma_gate = [None]

            def gated(ins):
                return ins

            def dma_pred(t):
                tv = pin.tile([128, 3 * R2], bf16, tag="tv")    # logq comps
                gated(nc.sync.dma_start(
                    sb(tv, 0, [[R2, 3], [1, R]]),
                    AP(pred_h, t * 128 * 3 * R, [[3 * R, 128], [1, 3 * R]])))
                tt = pin.tile([128, 3 * R2], bf16, tag="tt")    # t comps
                gated(nc.sync.dma_start(
                    sb(tt, 0, [[R2, 3], [1, R]]),
                    AP(pred_h, (NT + t) * 128 * 3 * R,
                       [[3 * R, 128], [1, 3 * R]])))
                state[t] = [tv, tt, None]

            def dma_targ(t):
                # T9 = [gtt(0:3C) | g1'(3C:6C) | gtq(6C:9C)]
                T9 = pin.tile([128, 9 * C], bf16, tag="T9")
                gated(nc.sync.dma_start(
                    sb(T9, 0, [[C, 3], [1, C]]),
                    AP(targ_h, t * 128 * 6 * C, [[6 * C, 128], [1, 3 * C]])))
                gated(nc.sync.dma_start(
                    sb(T9, 6 * C, [[C, 3], [1, C]]),
                    AP(targ_h, t * 128 * 6 * C + 3 * C,
                       [[6 * C, 128], [1, 3 * C]])))
                state[t][2] = T9

            def row_alloc(t):
                state[(t, 'row')] = (
                    pscr.tile([128, 3 * R2], bf16, tag=f"sq{t & 1}", name="sq"),
                    pscr.tile([128, R2], bf16, tag="n2a", name="n2a"),
                    pscr.tile([128, R2], bf16, tag="n2", name="n2"),
                    pscr.tile([128, R2], f32, tag="l", name="l"),
                    pscr.tile([128, R2], f32, tag="nh", name="nh"),
                    pscr.tile([128, R2], bf16, tag="i2n", name="i2n"),
                    pscr.tile([128, R2], bf16, tag="sh", name="sh"),
                    pscr.tile([128, R2], bf16, tag="ch", name="ch"),
                    pscr.tile([128, R2], bf16, tag="shsq", name="shsq"),
                    pscr.tile([128, R2], bf16, tag="sinn", name="sinn"),
                    prow.tile([128, 4 * R2], bf16, tag="AU", name="AU"),
                )

            def row_acts(t, lo=0, hi=R, g=''):
                tv = state[t][0]
                sq = state[(t, 'row')][0]
                n = hi - lo
                return mkact(t, ('rowLEb' + g) if t == 0 else 'rowLEa',
                             sb(sq, lo, [[R2, 3], [1, n]]),
                             sb(tv, lo, [[R2, 3], [1, n]]), AF.Square)

            def row_acts_le(t, lo=0, hi=R, g=''):
                l, nh, i2n = (state[(t, 'row')][k] for k in (3, 4, 5))
                # i2n = exp(-l/2) = 2/n   (l = ln(n^2/4))
                mkact(t, 'rowLEb' + g, i2n[:, lo:hi], l[:, lo:hi], AF.Exp,
                      scale=-0.5)

            def row_acts_tr(t, lo=0, hi=R, g=''):
                nh, sh, ch, shsq = (state[(t, 'row')][k] for k in (4, 6, 7, 8))
                mkact(t, 'rowTR' + g, sh[:, lo:hi], nh[:, lo:hi], AF.Sin)
                mkact(t, 'rowTR' + g, ch[:, lo:hi], nh[:, lo:hi], AF.Sin,
                      bias=PI2, scale=-1.0)                      # cos(n/2)
                mkact(t, 'rowTR' + g, shsq[:, lo:hi], sh[:, lo:hi], AF.Square)

            def row_vec_a(t, lo=0, hi=R, g=''):
                sq, n2a, n2, l, nh = state[(t, 'row')][0:5]
                TT(n2a[:, lo:hi], sq[:, lo:hi], sq[:, R2 + lo:R2 + hi],
                   OP.add)
                TT(n2[:, lo:hi], n2a[:, lo:hi],
                   sq[:, 2 * R2 + lo:2 * R2 + hi], OP.add)
                # n2 holds 2*|v|^2 (host scales logq by sqrt2);
                # l = ln(0.125*n2) = ln(n^2/4); nh = exp(l/2) = n/2
                mkact(t, 'rowLEb' + g, l[:, lo:hi], n2[:, lo:hi], AF.Ln,
                      scale=0.125)
                mkact(t, 'rowLEb' + g, nh[:, lo:hi], l[:, lo:hi], AF.Exp,
                      scale=0.5)

            def row_vec_b(t, lo=0, hi=R, g=''):
                tv = state[t][0]
                (sq, n2a, n2, l, nh, i2n, sh, ch, shsq, sinn, AU) = \
                    state[(t, 'row')]
                n = hi - lo
                TT(sinn[:, lo:hi], sh[:, lo:hi], ch[:, lo:hi], OP.mult)
                # A = sqrt2*cos(n) = sqrt2 - 2*sqrt2*sin^2(n/2)
                mkact(t, 'rowTR' + g, sb(AU, lo, [[1, n]]), shsq[:, lo:hi],
                      AF.Copy, bias=SQ2, scale=-2.0 * SQ2)
                # sn = sinn * i2n, in place (sn -> sinn tile)
                TT(sinn[:, lo:hi], sinn[:, lo:hi], i2n[:, lo:hi], OP.mult)
                # U = tv' * sin(n)/n = sqrt2 * v * sin(n)/n
                TT(sb(AU, R2 + lo, [[R2, 3], [1, n]]),
                   sb(tv, lo, [[R2, 3], [1, n]]),
                   sb(sinn, lo, [[0, 3], [1, n]]), OP.mult)
                state[(t, 'AU')] = AU

            def pair_rot(t):
                AU = state[(t, 'AU')]

                # P = AU@r0 * AU@r1 over 4 slabs -> qs2 tree
                P = pscr.tile([128, 4 * C], bf16, tag="P")
                TT(sb(P, 0, [[C, 4], [1, C]]),
                   sb(AU, 0, [[R2, 4], [1, C]]),
                   sb(AU, 1, [[R2, 4], [1, C]]), OP.mult)
                u = pscr.tile([128, 2 * C], bf16, tag="u", name="u")
                TT(sb(u, 0, [[C, 2], [1, C]]),
                   sb(P, 0, [[C, 2], [1, C]]),
                   sb(P, 2 * C, [[C, 2], [1, C]]), OP.add)
                qs2 = pscr.tile([128, C], bf16, tag="qs2", name="qs2")
                TT(qs2[:, :], u[:, 0:C], u[:, C:2 * C], OP.add)
                # clamp in place to +-1.9844 (4x-mode tensor_scalar), then
                # |qV|^2 = 4 - qs2^2 on ScalarE in f32
                nc.vector.tensor_scalar(qs2[:, :], qs2[:, :], QCLAMP,
                                        -QCLAMP, OP.min, OP.max)
                sqq = pscr.tile([128, C], f32, tag="sqq")
                mkact(t, 'pairLE', sqq[:, :], qs2[:, :], AF.Square)
                lq = pscr.tile([128, C], f32, tag="lq")
                mkact(t, 'pairLE', lq[:, :], sqq[:, :], AF.Ln,
                      bias=4.0, scale=-1.0)
                rs = pscr.tile([128, C], bf16, tag="rs")
                mkact(t, 'pairLE', rs[:, :], lq[:, :], AF.Exp, scale=-0.5)
                # r2 = qs2*rs emitted here so the arctan act can run early
                u2 = pscr.tile([128, 2 * C], bf16, tag="u", name="r2")
                TT(u2[:, 0:C], qs2[:, :], rs[:, :], OP.mult)
                at = pscr.tile([128, C], bf16, tag="at", name="at")
                mkact(t, 'pairTRa', at[:, :], u2[:, 0:C], AF.Arctan,
                      scale=-1.0)
                state[(t, 'at')] = at
                state[(t, 'rs')] = rs
                del qs2

                # PU12 slabs: 0-2 w1a, 3-5 cra, 6-8 w1b, 9-11 crb
                PU = pscr.tile([128, 12 * C], bf16, tag="PU")
                # w1a_j = A@r0 * U_j@r1 ; w1b_j = A@r1 * U_j@r0  (one instr)
                TT(sb(PU, 0, [[6 * C, 2], [C, 3], [1, C]]),
                   sb(AU, 0, [[1, 2], [0, 3], [1, C]]),
                   sb(AU, R2 + 1, [[-1, 2], [R2, 3], [1, C]]), OP.mult)
                # cra_c = U_{c+1}@r0 * U_{c+2}@r1 ; crb_c = U_{c+2}@r0 * U_{c+1}@r1
                TT(sb(PU, 3 * C, [[C, 2], [1, C]]),
                   sb(AU, 2 * R2, [[R2, 2], [1, C]]),
                   sb(AU, 3 * R2 + 1, [[-2 * R2, 2], [1, C]]), OP.mult)
                TT(sb(PU, 9 * C, [[C, 2], [1, C]]),
                   sb(AU, 3 * R2, [[-2 * R2, 2], [1, C]]),
                   sb(AU, 2 * R2 + 1, [[R2, 2], [1, C]]), OP.mult)
                TT(sb(PU, 5 * C, [[6 * C, 2], [1, C]]),
                   sb(AU, R2, [[R2, 2], [1, C]]),
                   sb(AU, 2 * R2 + 1, [[-R2, 2], [1, C]]), OP.mult)
                # [w1s|crs] = PU[0:6C] - PU[6C:12C]  (one 6C instr)
                S6 = pscr.tile([128, 6 * C], bf16, tag="S6")
                TT(sb(S6, 0, [[1, 6 * C]]),
                   sb(PU, 0, [[1, 6 * C]]),
                   sb(PU, 6 * C, [[1, 6 * C]]), OP.subtract)
                # qV = w1s - crs
                qV = pscr.tile([128, 3 * C], bf16, tag="qV")
                TT(sb(qV, 0, [[1, 3 * C]]),
                   sb(S6, 0, [[1, 3 * C]]),
                   sb(S6, 3 * C, [[1, 3 * C]]), OP.subtract)
                state[(t, 'rot')] = (qV, PU)

            def calc_d(t):
                tt = state[t][1]
                T9 = state[t][2]
                # d = t1 - t0 ; g1' = gtt' - d  (T9[3C:6C])
                d = pscr.tile([128, 3 * C], bf16, tag=f"d{t & 1}", name="d")
                TT(sb(d, 0, [[C, 3], [1, C]]),
                   sb(tt, 1, [[R2, 3], [1, C]]),
                   sb(tt, 0, [[R2, 3], [1, C]]), OP.subtract)
                TT(sb(T9, 3 * C, [[1, 3 * C]]),
                   sb(T9, 0, [[1, 3 * C]]),
                   sb(d, 0, [[1, 3 * C]]), OP.subtract)
                state[(t, 'd')] = d

            def pair_trans(t):
                tv, tt, T9 = state.pop(t)
                AU = state.pop((t, 'AU'))
                qV, PU = state.pop((t, 'rot'))
                d = state.pop((t, 'd'))

                def cross_into(dst, doff, v_t, v_pitch):
                    """dst[doff..doff+6C) gets the 6 cross products of
                    U@r0 x v."""
                    TT(sb(dst, doff, [[C, 2], [1, C]]),
                       sb(AU, 2 * R2, [[R2, 2], [1, C]]),
                       sb(v_t, 2 * v_pitch, [[-2 * v_pitch, 2], [1, C]]),
                       OP.mult)
                    TT(sb(dst, doff + 3 * C, [[C, 2], [1, C]]),
                       sb(AU, 3 * R2, [[-2 * R2, 2], [1, C]]),
                       sb(v_t, v_pitch, [[v_pitch, 2], [1, C]]), OP.mult)
                    TT(sb(dst, doff + 2 * C, [[3 * C, 2], [1, C]]),
                       sb(AU, R2, [[R2, 2], [1, C]]),
                       sb(v_t, v_pitch, [[-v_pitch, 2], [1, C]]), OP.mult)

                # b/cp live in the dead S6 buffer: b = S6[0:3C], cp = S6[3C:6C]
                B6 = pscr.tile([128, 6 * C], bf16, tag="S6", name="B6")
                # b = U0 x d   (cross scratch reuses PU[0:6C])
                cross_into(PU, 0, d, C)
                TT(sb(B6, 0, [[1, 3 * C]]),
                   sb(PU, 0, [[1, 3 * C]]),
                   sb(PU, 3 * C, [[1, 3 * C]]), OP.subtract)
                # cp = U0 x b   (cross scratch reuses PU[6C:12C])
                cross_into(PU, 6 * C, B6, C)
                TT(sb(B6, 3 * C, [[1, 3 * C]]),
                   sb(PU, 6 * C, [[1, 3 * C]]),
                   sb(PU, 9 * C, [[1, 3 * C]]), OP.subtract)
                # m = A0*b (borrows P[0:3C]) ; GX = [g2c | x12]
                m = pscr.tile([128, 4 * C], bf16, tag="P", name="m")
                TT(sb(m, 0, [[C, 3], [1, C]]),
                   sb(AU, 0, [[0, 3], [1, C]]),
                   sb(B6, 0, [[C, 3], [1, C]]), OP.mult)
                GX = pscr.tile([128, 6 * C], bf16, tag="GX")
                TT(sb(GX, 0, [[1, 3 * C]]),
                   sb(B6, 3 * C, [[1, 3 * C]]),
                   sb(m, 0, [[1, 3 * C]]), OP.subtract)
                state[(t, 'tail')] = (qV, T9, GX)

            def pair_tail(t):
                qV, T9, GX = state.pop((t, 'tail'))
                at = state.pop((t, 'at'))
                rs = state.pop((t, 'rs'))
                DUMP = pscr.tile([128, 6 * C], bf16, tag="DUMP", name="DUMP")
                last = t == NT - 1
                if last:
                    # dff_t/abs_t first: they do not need the ratio chain
                    acc1 = pacc.tile([128, 1], f32, tag=f"acc{t}a")
                    state[(t, 'dfft_ins')] = TT(
                       sb(GX, 0, [[1, 3 * C]]),
                       sb(GX, 0, [[1, 3 * C]]),
                       sb(T9, 3 * C, [[1, 3 * C]]), OP.subtract)
                    mkact(t, 'pairTRb', sb(DUMP, 0, [[1, 3 * C]]),
                          sb(GX, 0, [[1, 3 * C]]), AF.Abs,
                          accum_out=acc1[:, :])
                    accs.append(acc1)
                ratio = pscr.tile([128, C], bf16, tag="ratio")
                nc.vector.scalar_tensor_tensor(ratio[:, :], at[:, :], PI2,
                                               rs[:, :], OP.add, OP.mult)
                if not last:
                    # x12 = qV * ratio -> GX[3C:6C]; one 6C dff + abs
                    TT(sb(GX, 3 * C, [[C, 3], [1, C]]),
                       sb(qV, 0, [[C, 3], [1, C]]),
                       sb(ratio, 0, [[0, 3], [1, C]]), OP.mult)
                    TT(sb(GX, 0, [[1, 6 * C]]),
                       sb(GX, 0, [[1, 6 * C]]),
                       sb(T9, 3 * C, [[1, 6 * C]]), OP.subtract)
                    acc = pacc.tile([128, 1], f32, tag=f"acc{t}")
                    mkact(t, 'pairTRb', sb(DUMP, 0, [[1, 6 * C]]),
                          sb(GX, 0, [[1, 6 * C]]), AF.Abs, accum_out=acc[:, :])
                    accs.append(acc)
                else:
                    # q-side in column halves so ScalarE overlaps VectorE;
                    # false deps pin the DVE order (dff_t first) so the Abs
                    # chain drains while VectorE finishes the q side.
                    h = C // 2
                    prev = state.pop((t, 'dfft_ins'))
                    for ci, (lo, n) in enumerate(((0, h), (h, C - h))):
                        i_x = TT(sb(GX, 3 * C + lo, [[C, 3], [1, n]]),
                                 sb(qV, lo, [[C, 3], [1, n]]),
                                 sb(ratio, lo, [[0, 3], [1, n]]), OP.mult)
                        _add_dep_helper(i_x.ins, prev.ins, False,
                                        "tail order")
                        prev = TT(sb(GX, 3 * C + lo, [[C, 3], [1, n]]),
                                  sb(GX, 3 * C + lo, [[C, 3], [1, n]]),
                                  sb(T9, 6 * C + lo, [[C, 3], [1, n]]),
                                  OP.subtract)
                        accq = pacc.tile([128, 1], f32, tag=f"acc{t}q{ci}")
                        mkact(t, 'pairTRb', sb(DUMP, 3 * C + lo,
                                               [[C, 3], [1, n]]),
                              sb(GX, 3 * C + lo, [[C, 3], [1, n]]), AF.Abs,
                              accum_out=accq[:, :])
                        accs.append(accq)

            # ---- schedule: software-pipelined over tiles ----
            dummy = pacc.tile([128, 1], f32, tag="dummy")
            mkact(-1, 'init', dummy[:, :], nc.const_aps.aps[(f32, PI2)], AF.Ln)
            H = R // 2
            tv0 = pin.tile([128, 3 * R2], bf16, tag="tv")
            nc.sync.dma_start(
                sb(tv0, 0, [[R2, 3], [1, R]]),
                AP(pred_h, 0, [[3 * R, 128], [1, 3 * R]]))
            state[0] = [tv0, None, None]
            row_alloc(0)
            sqa = row_acts(0, 0, H, g='a')  # chunked head (one window)
            # every later DMA issues only after the first Square is running,
            # so tv(0) gets the full HBM bandwidth for the critical head
            dma_gate[0] = sqa
            tt0 = pin.tile([128, 3 * R2], bf16, tag="tt")
            gated(nc.sync.dma_start(
                sb(tt0, 0, [[R2, 3], [1, R]]),
                AP(pred_h, NT * 128 * 3 * R, [[3 * R, 128], [1, 3 * R]])))
            state[0][1] = tt0
            row_vec_a(0, 0, H, g='a')
            row_acts(0, H, R, g='a')
            row_vec_a(0, H, R, g='a')
            row_acts_le(0, g='a')
            dma_targ(0)
            calc_d(0)
            row_acts_tr(0, 0, H, g='a')
            row_vec_b(0, 0, H, g='a')
            row_acts_tr(0, H, R, g='a')
            row_vec_b(0, H, R, g='a')
            for t in range(NT):
                if t + 1 < NT:
                    dma_pred(t + 1)
                    row_alloc(t + 1)
                    row_acts(t + 1)
                pair_rot(t)
                if t + 1 < NT:
                    row_vec_a(t + 1)
                    row_acts_le(t + 1)
                pair_trans(t)
                if t + 1 < NT:
                    dma_targ(t + 1)
                    calc_d(t + 1)
                pair_tail(t)
                if t + 1 < NT:
                    row_acts_tr(t + 1)
                    row_vec_b(t + 1)

            # chain LUT activations across tiles: 6 table loads for NT=2.
            # rowTR(t+1) comes BEFORE pairTRa(t) so the next tile's row trig
            # (which gates its whole pair phase) is not queued behind this
            # tile's arctan chain; both share one trig table window.
            order = [('init', -1), ('rowLEba', 0), ('rowTRa', 0)]
            for t in range(NT - 1):
                order += [('rowLEa', t + 1), ('pairLE', t),
                          ('rowLEb', t + 1), ('rowTR', t + 1),
                          ('pairTRa', t), ('pairTRb', t)]
            order += [('pairLE', NT - 1), ('pairTRa', NT - 1),
                      ('pairTRb', NT - 1)]
            seq = []
            for gname, ti in order:
                seq.extend(groups.get((ti, gname), []))
            for i in range(1, len(seq)):
                _add_dep_helper(seq[i].ins, seq[i - 1].ins, False,
                                "act table-set grouping")

            for idx, acc in enumerate(accs):
                nc.sync.dma_start(AP(out_h, idx, [[4, 128], [1, 1]]),
                                  acc[:, :])

    nc.compile()
    return nc


def _get_nc():
    if "nc" not in _BUILT:
        _BUILT["nc"] = _build()
    return _BUILT["nc"]


def run_device(pred, targ, srx, srq, trace=False):
    """-> (sum_merged, exec_ns); t-lanes pre-scaled by k = exp(srq-srx)."""
    import ml_dtypes
    from concourse.bass_utils import run_bass_kernel_spmd

    bf16 = ml_dtypes.bfloat16
    nc = _get_nc()
    srx_v = float(np.asarray(srx).reshape(-1)[0])
    srq_v = float(np.asarray(srq).reshape(-1)[0])
    k = float(np.exp(srq_v - srx_v))

    p = np.asarray(pred, dtype=np.float32).reshape(-1, 6).copy()
    g = np.asarray(targ, dtype=np.float32).reshape(-1, 6).copy()
    p[:, :3] *= k
    p[:, 3:] *= np.sqrt(np.float32(2.0))
    g[:, :3] *= k
    n_dup = ROWS_PAD - p.shape[0]
    p_pad = np.concatenate([p, np.repeat(p[-1:], n_dup, axis=0)], axis=0)
    g_pad = np.concatenate(
        [g, np.zeros((PAIRS_PAD - g.shape[0], 6), np.float32)], axis=0)
    p_pad = p_pad.astype(bf16)
    g_pad = g_pad.astype(bf16)

    # per-(tile,partition) contiguous blocks:
    # pred = [tv(t) blocks][tt(t) blocks]; block[t] = [128, 3, R] row-major
    # targ = [t blocks]; block[t] = [128, 6, C]
    IR = ((np.arange(128) * D)[None, :, None]
          + (np.arange(NT) * C)[:, None, None]
          + np.arange(R)[None, None, :])           # [NT,128,R]
    IC = ((np.arange(128) * D)[None, :, None]
          + (np.arange(NT) * C)[:, None, None]
          + np.arange(C)[None, None, :])           # [NT,128,C]
    in_maps = []
    for c in range(N_CORES):
        s = c * PPC
        pc = p_pad[s:s + PPC + 1]                  # [PPC+1, 6] bf16
        gc = g_pad[s:s + PPC]                      # [PPC, 6]
        tv = np.stack([pc[:, 3 + j][IR] for j in range(3)], axis=2)
        tt = np.stack([pc[:, j][IR] for j in range(3)], axis=2)
        tg = np.stack([gc[:, j][IC] for j in range(6)], axis=2)
        pred_buf = np.concatenate(
            [tv.reshape(-1), tt.reshape(-1)])      # [NT*128*3R * 2]
        in_maps.append({
            "pred": np.ascontiguousarray(pred_buf),
            "targ": np.ascontiguousarray(tg.reshape(-1)),
        })
    res = run_bass_kernel_spmd(nc, in_maps, core_ids=list(range(N_CORES)),
                               trace=trace)
    psum = np.stack([np.asarray(res.results[i]["out"], dtype=np.float64)
                     for i in range(N_CORES)])
    s_all = float(psum[:, :, 0:4].sum())
    return s_all, res.exec_time_ns


def kernel(pred, targ, srx, srq):
    trace = bool(int(os.environ.get("VO_KERNEL_TRACE", "0")))
    s_all, _ = run_device(pred, targ, srx, srq, trace=trace)
    if s_all < 1.0:
        # cold-device flake (all-zero readback) -- run again
        s_all, _ = run_device(pred, targ, srx, srq, trace=trace)
    srx_v = float(np.asarray(srx).reshape(-1)[0])
    srq_v = float(np.asarray(srq).reshape(-1)[0])
    out = (np.exp(-srq_v) * s_all / (3.0 * NPAIRS) + srx_v + srq_v)
    return np.array([out], dtype=np.float32)
